# revision 1
# baseline (speedup 1.0000x reference)
"""MoE (top-2 of 8 experts) Trainium2 kernel, expert-parallel over 8 NeuronCores.

Changes vs v2:
  - Gate exchange via AllToAll of the per-shard combine weights in bf16
    ([E, SHARD] rows -> each core receives its expert's weight for all N
    tokens, flat in token order).  No AllGather, no [N, E] comb loads.
  - Routing in wrap-128 layout [128, 64] (token n at [n%128, n//128]),
    loaded with one XBAR transpose DMA.  Prefix over partitions via a
    strict-lower matmul, cross-column scan on DVE, then an 8-matmul
    fan-out converts dest to the scatter ucode's wrap-16 [128, 512]
    index layout (replicated across cores by construction).
  - Inverse-permutation dispatch: dma_scatter_add writes 256-byte
    token-id rows into islot[g] (640x128 i16); per pass the compact ids
    are read back, converted to wrap-16 gather indices, and xt is
    gathered directly from xbf with dma_gather(transpose=True).  The
    16.8 MB x row scatter and its zero-init disappear.
  - Single dump row (CAP_G) per group; y_disp zero-init is 4 rows.
  - Per-pass xt gather + index prep are emitted before the previous
    group's un-dispatch so they don't queue behind it on SWDGE.
"""

import numpy as np
import ml_dtypes

import concourse.bass as bass
import concourse.tile as tile
from concourse import bacc, mybir
from concourse.masks import make_identity

FP32 = mybir.dt.float32
BF16 = mybir.dt.bfloat16
I16 = mybir.dt.int16
Alu = mybir.AluOpType
Act = mybir.ActivationFunctionType


class Cfg:
    def __init__(self, N=8192, D=1024, F=4096, E=8, CAP_G=576, NGROUP=4, CHUNK=512):
        self.N, self.D, self.F, self.E = N, D, F, E
        self.CAP_G = CAP_G          # compact slots per token group (= pass width)
        self.NGROUP = NGROUP
        self.CHUNK = CHUNK          # un-dispatch token chunk
        self.NCORE = 8
        self.DC = D // 128
        self.FC = F // 128
        self.GTOK = N // NGROUP
        self.SHARD = N // self.NCORE
        self.ST = self.SHARD // 128
        self.NCHUNK = N // CHUNK
        self.CPG = self.NCHUNK // NGROUP
        self.SPC = CHUNK // 128
        self.GW = ((CAP_G + 127) // 128) * 128   # gather width (pad idxs -> 0)
        self.XROWS = CAP_G + CHUNK               # incl. per-token dump rows
        self.WCOL = N // 128                     # wrap-128 columns (64)
        self.GCOL = self.WCOL // NGROUP          # wrap-128 cols per group (16)
        assert CAP_G % 64 == 0 and N % CHUNK == 0 and CHUNK % 128 == 0


def host_inputs(cfg: Cfg, x, Wg, bg, W1, b1, W2, b2):
    """Build the 8 per-core input maps (numpy only, layout + dtype cast)."""
    c = cfg
    xf = np.ascontiguousarray(np.asarray(x, np.float32).reshape(c.N, c.D))
    Wg = np.ascontiguousarray(np.asarray(Wg, np.float32))
    bg = np.asarray(bg, np.float32).reshape(1, c.E)
    bgr = np.ascontiguousarray(np.broadcast_to(bg, (128, c.E)))
    W1 = np.asarray(W1)
    W2 = np.asarray(W2)
    b1 = np.asarray(b1, np.float32)
    b2 = np.asarray(b2, np.float32)
    xbf = xf.astype(ml_dtypes.bfloat16)

    # strict-lower [128, 128] for the within-column (partition) prefix
    k = np.arange(128)[:, None]
    i = np.arange(128)[None, :]
    stri128 = (k < i).astype(np.float32)

    # [16, 128] replication matrix: rep16[k, m] = (m % 16 == k)
    rep16 = (np.arange(128)[None, :] % 16 == np.arange(16)[:, None]).astype(
        np.float32)

    # fan-out selectors: selq[k, q, m] = (k == q*16 + m%16)
    kk = np.arange(128)[:, None, None]
    qq = np.arange(8)[None, :, None]
    mm = np.arange(128)[None, None, :]
    selq = (kk == qq * 16 + mm % 16).astype(np.float32)

    # distinct dump row per token-in-chunk: 576 + (n % CHUNK), n = s*128 + w
    ww = np.arange(128)[:, None]
    sc = np.arange(c.WCOL)[None, :]
    dumppc = (c.CAP_G + (sc % (c.CHUNK // 128)) * 128 + ww).astype(np.float32)

    # gather-transpose iota idxs: column j = s*16 + (p%16) -> row j (pad -> 0)
    ss = np.arange(c.GW // 16)[None, :]
    jj = ss * 16 + (np.arange(128)[:, None] % 16)
    iota_x = np.ascontiguousarray(
        np.where(jj < c.CAP_G, jj, 0).astype(np.int16))

    maps = []
    for e in range(c.NCORE):
        w1t = W1[e].astype(ml_dtypes.bfloat16).reshape(
            c.DC, 128, c.FC, 128).transpose(2, 1, 0, 3).reshape(c.FC, 128, c.D)
        w2t = W2[e].astype(ml_dtypes.bfloat16).reshape(
            c.FC, 128, c.D).transpose(1, 0, 2).reshape(128, c.FC * c.D)
        b2r = np.broadcast_to(b2[e].reshape(1, c.D), (128, c.D))
        maps.append({
            "xshard": np.ascontiguousarray(xf[e * c.SHARD:(e + 1) * c.SHARD]),
            "xbf": xbf,
            "wg": Wg,
            "bgr": bgr,
            "w1s": np.ascontiguousarray(w1t),
            "w2s": np.ascontiguousarray(w2t),
            "b1v": np.ascontiguousarray(b1[e]),
            "b2r": np.ascontiguousarray(b2r),
            "stri128": stri128,
            "rep16": rep16,
            "selq": np.ascontiguousarray(selq),
            "iotax": iota_x,
            "dumppc": np.ascontiguousarray(dumppc),
        })
    return maps


def assemble(cfg: Cfg, results):
    c = cfg
    S = c.GTOK // c.NCORE
    out = np.empty((c.N, c.D), np.float32)
    for e in range(c.NCORE):
        o = np.asarray(results[e]["out"], np.float32)
        for q in range(c.NGROUP):
            out[q * c.GTOK + e * S: q * c.GTOK + (e + 1) * S] = o[q * S:(q + 1) * S]
    return out


def build(cfg: Cfg, debug: bool = False):
    c = cfg
    nc = bacc.Bacc(
        "TRN2", target_bir_lowering=False, debug=debug,
        enable_asserts=True, num_devices=c.NCORE,
    )

    xshard = nc.dram_tensor("xshard", [c.SHARD, c.D], FP32, kind="ExternalInput").ap()
    xbf = nc.dram_tensor("xbf", [c.N, c.D], BF16, kind="ExternalInput").ap()
    wg = nc.dram_tensor("wg", [c.D, c.E], FP32, kind="ExternalInput").ap()
    bgr = nc.dram_tensor("bgr", [128, c.E], FP32, kind="ExternalInput").ap()
    w1s = nc.dram_tensor("w1s", [c.FC, 128, c.D], BF16, kind="ExternalInput").ap()
    w2s = nc.dram_tensor("w2s", [128, c.FC * c.D], BF16, kind="ExternalInput").ap()
    b1v = nc.dram_tensor("b1v", [c.F], FP32, kind="ExternalInput").ap()
    b2r = nc.dram_tensor("b2r", [128, c.D], FP32, kind="ExternalInput").ap()
    stri = nc.dram_tensor("stri128", [128, 128], FP32, kind="ExternalInput").ap()
    rep16 = nc.dram_tensor("rep16", [16, 128], FP32, kind="ExternalInput").ap()
    selq = nc.dram_tensor("selq", [128, 8, 128], FP32, kind="ExternalInput").ap()
    iotax = nc.dram_tensor("iotax", [128, c.GW // 16], I16,
                           kind="ExternalInput").ap()
    dumppc = nc.dram_tensor("dumppc", [128, c.WCOL], FP32,
                            kind="ExternalInput").ap()
    out_ext = nc.dram_tensor("out", [c.SHARD, c.D], FP32, kind="ExternalOutput").ap()

    RG = [list(range(c.NCORE))]

    import contextlib
    with tile.TileContext(nc) as tc, contextlib.ExitStack() as stk:
        if True:
            pool = lambda *a, **k: stk.enter_context(tc.tile_pool(*a, **k))
            consts = pool(name="consts", bufs=1)
            w2res = pool(name="w2res", bufs=1)
            w1p = pool(name="w1p", bufs=9)
            dram = pool(name="dram", bufs=1, space="DRAM")
            shared = pool(name="shared", bufs=1, space="DRAM")
            acts = pool(name="acts", bufs=1)
            xtp = pool(name="xtp", bufs=2)
            xcp = pool(name="xcp", bufs=2)
            yout = pool(name="yout", bufs=2)
            udp = pool(name="udp", bufs=2)
            route = pool(name="route", bufs=1)
            psum = pool(name="psum", bufs=2, space="PSUM")
            psum1b = pool(name="psum1b", bufs=2, space="PSUM")
            psum2a = pool(name="psum2a", bufs=1, space="PSUM")
            psum2b = pool(name="psum2b", bufs=1, space="PSUM")

            # ---------- constants ----------
            ident = consts.tile([128, 128], FP32)
            make_identity(nc, ident[:])
            ident_bf = consts.tile([128, 128], BF16)
            nc.vector.tensor_copy(ident_bf[:], ident[:])
            stri_sb = consts.tile([128, 128], FP32)
            nc.scalar.dma_start(stri_sb[:], stri)
            rep_sb = consts.tile([16, 128], FP32)
            nc.scalar.dma_start(rep_sb[:], rep16)
            selq_sb = consts.tile([128, 8, 128], FP32)
            nc.scalar.dma_start(selq_sb[:], selq)
            ones128 = consts.tile([128, 1], FP32)
            nc.vector.memset(ones128[:], 1.0)
            onesr = consts.tile([1, 128], FP32)
            nc.vector.memset(onesr[:], 1.0)
            bg_sb = consts.tile([128, c.E], FP32)
            nc.scalar.dma_start(bg_sb[:], bgr)
            wg_sb = consts.tile([128, c.DC, c.E], FP32)
            nc.scalar.dma_start(wg_sb[:], wg.rearrange("(a p) e -> p a e", p=128))
            b1_sb = consts.tile([128, c.FC], FP32)
            nc.scalar.dma_start(b1_sb[:], b1v.rearrange("(a p) -> p a", p=128))
            b2_sb = consts.tile([128, c.D], FP32)
            nc.scalar.dma_start(b2_sb[:], b2r)
            ztb = consts.tile([128, c.D], BF16)
            nc.vector.memset(ztb[:], 0.0)
            zti = consts.tile([128, 128], I16)
            nc.vector.memset(zti[:], 0)
            iota_sb = consts.tile([128, c.GW // 16], I16)
            nc.scalar.dma_start(iota_sb[:], iotax)
            dump_sb = consts.tile([128, c.WCOL], FP32)
            nc.scalar.dma_start(dump_sb[:], dumppc)

            # ---------- scratch DRAM ----------
            x_disp = [dram.tile([c.XROWS, c.D], BF16, name=f"xdisp{g}")
                      for g in range(c.NGROUP)]
            y_disp = [dram.tile([c.XROWS, c.D], BF16, name=f"ydisp{g}")
                      for g in range(c.NGROUP)]
            rs_in = [dram.tile([c.GTOK, c.D], BF16, name=f"rsin{g}")
                     for g in range(c.NGROUP)]
            rs_out = [dram.tile([c.GTOK // c.NCORE, c.D], BF16, name=f"rsout{g}")
                      for g in range(c.NGROUP)]
            comb_t = dram.tile([c.E * c.ST, 128], BF16, name="combt")
            wselbf = dram.tile([c.N // 128, 128], BF16, name="wselbf")
            # wselbf is [64, 128] bf16 = flat wsel[n] at [n//128, n%128]

            # zero-init: x_disp compact (scatter-add); y_disp dump region
            def zero_rows(t, r0, r1):
                r = r0
                while r < r1:
                    h = min(128, r1 - r)
                    nc.scalar.dma_start(t[r:r + h, :], ztb[:h, :])
                    r += h

            for g in range(c.NGROUP):
                zero_rows(x_disp[g], 0, c.CAP_G)
                zero_rows(y_disp[g], c.CAP_G, c.XROWS)

            # resident W2 [128, FC, D]
            w2sb = w2res.tile([128, c.FC, c.D], BF16)
            nc.scalar.dma_start(
                w2sb[:], w2s.rearrange("p (f d) -> p f d", f=c.FC))

            # ---------- phase 1: gate over own shard (fp32) ----------
            with contextlib.ExitStack() as gstk:
                gpool = lambda *a, **k: gstk.enter_context(tc.tile_pool(*a, **k))
                gate = gpool(name="gate", bufs=1)
                gld = gpool(name="gld", bufs=2)
                gxt = gpool(name="gxt", bufs=2)
                psg = gpool(name="psg", bufs=2, space="PSUM")

                lgall = gate.tile([128, c.ST, c.E], FP32)
                for st in range(c.ST):
                    xs = gld.tile([128, c.D], FP32, tag="xs")
                    nc.sync.dma_start(xs[:], xshard[128 * st:128 * (st + 1), :])
                    xts = gxt.tile([128, c.DC, 128], FP32, tag="xts")
                    for d in range(c.DC):
                        pt = psg.tile([128, 128], FP32, tag="tr")
                        nc.tensor.transpose(pt[:], xs[:, 128 * d:128 * (d + 1)],
                                            ident[:])
                        nc.vector.tensor_copy(xts[:, d, :], pt[:])
                    pl = psg.tile([128, c.E], FP32, tag="tr")
                    for d in range(c.DC):
                        nc.tensor.matmul(
                            pl[:], lhsT=xts[:, d, :], rhs=wg_sb[:, d, :],
                            start=(d == 0), stop=(d == c.DC - 1))
                    nc.vector.tensor_copy(lgall[:, st, :], pl[:])
                nc.vector.tensor_tensor(
                    out=lgall[:], in0=lgall[:],
                    in1=bg_sb[:, None, :].to_broadcast([128, c.ST, c.E]),
                    op=Alu.add)
                mxall = gate.tile([128, c.ST, 8], FP32)
                for st in range(c.ST):
                    nc.vector.max(out=mxall[:, st, :], in_=lgall[:, st, :])
                wsig = gate.tile([128, c.ST, 1], FP32)
                nc.vector.tensor_tensor(
                    out=wsig[:], in0=mxall[:, :, 0:1], in1=mxall[:, :, 1:2],
                    op=Alu.subtract)
                nc.scalar.activation(wsig[:], wsig[:], Act.Sigmoid)
                w2sig = gate.tile([128, c.ST, 1], FP32)
                nc.vector.tensor_scalar(
                    out=w2sig[:], in0=wsig[:], scalar1=-1.0, scalar2=1.0,
                    op0=Alu.mult, op1=Alu.add)
                m1 = gate.tile([128, c.ST, c.E], FP32)
                nc.vector.tensor_tensor(
                    out=m1[:], in0=lgall[:],
                    in1=mxall[:, :, 0:1].to_broadcast([128, c.ST, c.E]),
                    op=Alu.is_equal)
                msk = gate.tile([128, c.ST, c.E], FP32)
                nc.vector.tensor_scalar_mul(msk[:], m1[:], 1e30)
                nc.vector.tensor_tensor(
                    out=msk[:], in0=lgall[:], in1=msk[:], op=Alu.subtract)
                m2 = gate.tile([128, c.ST, c.E], FP32)
                nc.vector.tensor_tensor(
                    out=m2[:], in0=msk[:],
                    in1=mxall[:, :, 1:2].to_broadcast([128, c.ST, c.E]),
                    op=Alu.is_equal)
                cmb = gate.tile([128, c.ST, c.E], FP32)
                nc.vector.tensor_tensor(
                    out=cmb[:], in0=m1[:],
                    in1=wsig[:].to_broadcast([128, c.ST, c.E]), op=Alu.mult)
                nc.vector.tensor_tensor(
                    out=m2[:], in0=m2[:],
                    in1=w2sig[:].to_broadcast([128, c.ST, c.E]), op=Alu.mult)
                nc.vector.tensor_tensor(
                    out=cmb[:], in0=cmb[:], in1=m2[:], op=Alu.add)
                # reorder to [128, (e st)] bf16, PE transpose -> [64, 128]
                cmbb = gate.tile([128, c.E, c.ST], BF16)
                nc.vector.tensor_copy(
                    cmbb[:], cmb[:].rearrange("p s e -> p e s"))
                ptc = psg.tile([c.E * c.ST, 128], BF16, tag="tr")
                nc.tensor.transpose(
                    ptc[:], cmbb[:].rearrange("p e s -> p (e s)"), ident_bf[:])
                t2s = gate.tile([c.E * c.ST, 128], BF16)
                nc.vector.tensor_copy(t2s[:], ptc[:])
                nc.sync.dma_start(comb_t[:], t2s[:])

            nc.gpsimd.collective_compute(
                "AllToAll", Alu.bypass,
                ins=[comb_t[:]], outs=[wselbf[:]], replica_groups=RG,
            )

            # ---------- phase 2: routing (wrap-128 [128, 64] layout) ----------
            wsel_t = route.tile([128, c.WCOL], BF16)
            wselgp = route.tile([128, c.WCOL], FP32)
            dest_rep = route.tile([128, c.N // 16], I16)
            with contextlib.ExitStack() as rstk:
                rpool = lambda *a, **k: rstk.enter_context(tc.tile_pool(*a, **k))
                rtmp = rpool(name="rtmp", bufs=1)
                psr = rpool(name="psr", bufs=1, space="PSUM")

                nc.sync.dma_start(wsel_t[:], wselbf[:], transpose=True)
                nc.vector.tensor_copy(wselgp[:], wsel_t[:])
                m = rtmp.tile([128, c.WCOL], FP32)
                nc.vector.tensor_scalar(
                    out=m[:], in0=wselgp[:], scalar1=0.0, scalar2=None,
                    op0=Alu.is_gt)
                # within-column exclusive prefix over partitions
                ppos = psr.tile([128, c.WCOL], FP32, tag="ppos")
                nc.tensor.matmul(ppos[:], lhsT=stri_sb[:], rhs=m[:],
                                 start=True, stop=False)
                # column totals
                pcs = psr.tile([1, c.WCOL], FP32, tag="r2")
                nc.tensor.matmul(pcs[:], lhsT=ones128[:], rhs=m[:],
                                 start=True, stop=True)
                cs = rtmp.tile([1, c.WCOL], FP32)
                nc.vector.tensor_copy(cs[:], pcs[:])
                # per-group exclusive scan of column sums
                csx = rtmp.tile([1, c.WCOL], FP32)
                for q in range(c.NGROUP):
                    sl = slice(c.GCOL * q, c.GCOL * (q + 1))
                    nc.vector.tensor_tensor_scan(
                        out=csx[:, sl], data0=cs[:, sl], data1=cs[:, sl],
                        initial=0.0, op0=Alu.add, op1=Alu.bypass)
                nc.vector.tensor_tensor(
                    out=csx[:], in0=csx[:], in1=cs[:], op=Alu.subtract)
                nc.tensor.matmul(ppos[:], lhsT=onesr[:], rhs=csx[:],
                                 start=False, stop=True)
                pos = rtmp.tile([128, c.WCOL], FP32)
                nc.vector.tensor_copy(pos[:], ppos[:])
                # dest = m ? pos : dump (per-token dump rows)
                dest_f = rtmp.tile([128, c.WCOL], FP32)
                nmw = rtmp.tile([128, c.WCOL], FP32)
                nc.vector.tensor_scalar(
                    out=nmw[:], in0=m[:], scalar1=-1.0, scalar2=1.0,
                    op0=Alu.mult, op1=Alu.add)
                nc.vector.tensor_tensor(
                    out=nmw[:], in0=dump_sb[:], in1=nmw[:], op=Alu.mult)
                nc.vector.tensor_tensor(
                    out=dest_f[:], in0=pos[:], in1=m[:], op=Alu.mult)
                nc.vector.tensor_tensor(
                    out=dest_f[:], in0=dest_f[:], in1=nmw[:], op=Alu.add)
                # fan-out to wrap-16 [128, 512] (replicated across cores)
                pfan = psr.tile([128, c.N // 16], FP32, tag="r2")
                for q in range(8):
                    nc.tensor.matmul(
                        pfan[:, c.WCOL * q:c.WCOL * (q + 1)],
                        lhsT=selq_sb[:, q, :], rhs=dest_f[:],
                        start=True, stop=True)
                nc.vector.tensor_copy(
                    dest_rep[:].rearrange("p (s q) -> p s q", q=8),
                    pfan[:].rearrange("p (q s) -> p s q", q=8))

            # ---------- phase 3: dispatch (scatter bf16 x rows) ----------
            for ch in range(c.NCHUNK):
                xc = xcp.tile([128, c.SPC, c.D], BF16, tag="xc")
                nc.sync.dma_start(
                    xc[:],
                    xbf[c.CHUNK * ch:c.CHUNK * (ch + 1), :]
                    .rearrange("(s p) d -> p s d", p=128))
                nc.gpsimd.dma_scatter_add(
                    out_ap=x_disp[ch // c.CPG][:],
                    in_ap=xc[:],
                    idxs_ap=dest_rep[:, (c.CHUNK // 16) * ch:
                                     (c.CHUNK // 16) * (ch + 1)],
                    num_idxs=c.CHUNK, num_idxs_reg=c.CHUNK,
                    elem_size=c.D)

            # ---------- phase 4/5: FFN passes + un-dispatch + RS ----------
            W = c.CAP_G
            NT = (W + 127) // 128
            xt_tiles = {}

            def prep_pass(g):
                """Gather-transpose x_disp[g] compact rows -> xt."""
                xt = xtp.tile([128, c.DC, c.GW], BF16, tag="xt")
                nc.gpsimd.dma_gather(
                    out_ap=xt[:], in_ap=x_disp[g][:], idxs_ap=iota_sb[:],
                    num_idxs=c.GW, num_idxs_reg=c.GW, elem_size=c.D,
                    transpose=True)
                xt_tiles[g] = xt

            def ffn_pass(g):
                xt = xt_tiles.pop(g)
                ht = acts.tile([128, c.FC, W], BF16, tag="ht")
                for f in range(c.FC):
                    w1t = w1p.tile([128, c.DC, 128], BF16, tag="w1t")
                    nc.scalar.dma_start(w1t[:], w1s[f])
                    p1 = psum.tile([128, 512], FP32, tag="mm1")
                    p1b = psum1b.tile([128, W - 512], FP32, tag="mm1b")
                    for d in range(c.DC):
                        nc.tensor.matmul(
                            p1[:, :], lhsT=w1t[:, d, :],
                            rhs=xt[:, d, 0:512],
                            start=(d == 0), stop=(d == c.DC - 1))
                        nc.tensor.matmul(
                            p1b[:, :], lhsT=w1t[:, d, :],
                            rhs=xt[:, d, 512:W],
                            start=(d == 0), stop=(d == c.DC - 1))
                    nc.scalar.activation(
                        ht[:, f, 0:512], p1[:], Act.Gelu, bias=b1_sb[:, f:f + 1])
                    nc.scalar.activation(
                        ht[:, f, 512:W], p1b[:], Act.Gelu,
                        bias=b1_sb[:, f:f + 1])
                # prefetch next pass's idx/xt ahead of the un-dispatch below
                if g + 1 < c.NGROUP:
                    prep_pass(g + 1)
                for tb in range(NT):
                    t0 = 128 * tb
                    nr = min(128, W - t0)
                    ysb = yout.tile([128, c.D], BF16, tag="ysb")
                    p2a = psum2a.tile([128, 512], FP32, tag="mm2a")
                    p2b = psum2b.tile([128, 512], FP32, tag="mm2b")
                    for f in range(c.FC):
                        nc.tensor.matmul(
                            p2a[:nr, :], lhsT=ht[:, f, t0:t0 + nr],
                            rhs=w2sb[:, f, 0:512],
                            start=(f == 0), stop=(f == c.FC - 1))
                        nc.tensor.matmul(
                            p2b[:nr, :], lhsT=ht[:, f, t0:t0 + nr],
                            rhs=w2sb[:, f, 512:1024],
                            start=(f == 0), stop=(f == c.FC - 1))
                    nc.vector.tensor_tensor(
                        out=ysb[:nr, 0:512], in0=p2a[:nr, :],
                        in1=b2_sb[:nr, 0:512], op=Alu.add)
                    nc.vector.tensor_tensor(
                        out=ysb[:nr, 512:1024], in0=p2b[:nr, :],
                        in1=b2_sb[:nr, 512:1024], op=Alu.add)
                    nc.sync.dma_start(y_disp[g][t0:t0 + nr, :], ysb[:nr, :])

            def undisp_rs(g):
                for cc in range(c.CPG):
                    ch = g * c.CPG + cc
                    ud = udp.tile([128, c.SPC, c.D], BF16, tag="ud")
                    nc.gpsimd.dma_gather(
                        out_ap=ud[:],
                        in_ap=y_disp[g][:],
                        idxs_ap=dest_rep[:, (c.CHUNK // 16) * ch:
                                         (c.CHUNK // 16) * (ch + 1)],
                        num_idxs=c.CHUNK, num_idxs_reg=c.CHUNK,
                        elem_size=c.D)
                    for s in range(c.SPC):
                        nc.vector.tensor_scalar_mul(
                            ud[:, s, :], ud[:, s, :],
                            wselgp[:, c.SPC * ch + s:c.SPC * ch + s + 1])
                    nc.scalar.dma_start(
                        rs_in[g][c.CHUNK * cc:c.CHUNK * (cc + 1), :]
                        .rearrange("(s p) d -> p s d", p=128),
                        ud[:])
                nc.gpsimd.collective_compute(
                    "ReduceScatter", Alu.add,
                    ins=[rs_in[g][:]], outs=[rs_out[g][:]], replica_groups=RG,
                )
                S = c.GTOK // c.NCORE
                nc.gpsimd.dma_start(out_ext[S * g:S * (g + 1), :],
                                    rs_out[g][:])

            prep_pass(0)
            for g in range(c.NGROUP):
                ffn_pass(g)
                if g >= 1:
                    undisp_rs(g - 1)
            undisp_rs(c.NGROUP - 1)

    nc.compile()
    return nc


def run(x, Wg, bg, W1, b1, W2, b2, trace=False, **spmd_kwargs):
    from concourse.bass_utils import run_bass_kernel_spmd
    cfg = Cfg()
    B, T, D = np.asarray(x).shape
    assert (B * T, D) == (cfg.N, cfg.D)
    nc = build(cfg, debug=False)
    in_maps = host_inputs(cfg, x, Wg, bg, W1, b1, W2, b2)
    res = run_bass_kernel_spmd(nc, in_maps, core_ids=list(range(cfg.NCORE)),
                               trace=trace, **spmd_kwargs)
    out = assemble(cfg, res.results)
    return out.reshape(B, T, D), res


def kernel(x, Wg, bg, W1, b1, W2, b2, top_k):
    assert int(top_k) == 2
    out, _ = run(x, Wg, bg, W1, b1, W2, b2, trace=False)
    return out

